# revision 19
# baseline (speedup 1.0000x reference)
"""CTC loss on 8 Trainium2 NeuronCores (Bass/Tile).

Strategy (data parallel, per the sharding hint): batch B=64 is split 8
samples/core. The host gathers each sample's distinct lattice emission rows
(1 blank + 30 labels = 31 "slots" per sample) from log_probs, max-normalizes
them, and ships only those ~1.5MB (fp8) to the device — never the 170MB
log-prob tensor. Each core runs the CTC forward recurrence in linear space:

  - lattice rows computed as first-order scans over t (tensor_tensor_scan),
  - T split into C=16 chunks mapped to SBUF partitions (lanes = (b, c)),
    cross-chunk carries solved exactly with per-slot transfer matrices G
    built on the PE/ACT from bulk chunk-sum cumulants,
  - per-(sample,chunk) static log offsets Lam keep all stored values in
    fp32 range; the stitch algebra folds the offsets in exactly, so they
    do not affect the result.  Lam is estimated ON DEVICE by a coarse
    2-step-windowed lattice DP run in LINEAR space with a max-renorm
    every window (256 sequential steps over an [8, 61] state, all on the
    DVE so the serial chain never pays cross-engine semaphore hops; the
    applied renorm scales are logged and prefix-summed at the end, which
    recovers the levels exactly).  Its input is derived on device from
    pair-products of the already-resident exp(z), moved from (b,c)-lane
    partitions to sample partitions by one SBUF->SBUF reshape DMA.

Each core returns the 8 lattice rows its samples actually end in plus its
Lam estimates: a (8, 115) f32 output per core. Per-sample losses are
reconstructed on host from that, then averaged (the "all-reduce").

Perf structure: the dominant cost is the fixed per-call axon-tunnel round
trip (~60-80ms); everything else is arranged to add as little as possible
on top of it.  The PJRT executable is lowered+compiled ONCE and cached
in-process; per call we only feed new (already concat-shaped) numpy
arrays (~1.15MB: fp8 emissions + f32 norm cumulants + u8 selectors).
The targets-static u8 constants are pushed to the devices once and the
resident jax.Array is reused.  The host prep is one fused XLA-CPU jit
(gather + max-normalize + fp8 cast) emitting transfer-ready arrays.
"""
import os
import tempfile

import numpy as np

import jax

jax.config.update("jax_compilation_cache_dir",
                  os.path.join(tempfile.gettempdir(), "bass_jax_cache"))
jax.config.update("jax_persistent_cache_min_entry_size_bytes", 0)
jax.config.update("jax_persistent_cache_min_compile_time_secs", 0.0)

import concourse.bacc as bacc
import concourse.tile as tile
from concourse import mybir
from concourse import bass2jax

F32 = mybir.dt.float32
F8 = mybir.dt.float8e4
U8 = mybir.dt.uint8

T, B, V, S = 512, 64, 1296, 30
L = 2 * S + 1          # 61 lattice rows
NS = S + 1             # 31 distinct emission slots (slot 0 = blank)
C = 16                 # time chunks  (lanes = 8 local samples x 16 chunks)
TC = T // C            # 32 steps per chunk
NCORES = 8
BLOC = B // NCORES     # 8 samples per core
NLANE = NCORES * 128   # concat partition rows across cores = 1024
BLANK = 0
OW = 2 * (TC + 1) + 1 + TC   # packed output: 2 rows + baseM + cumM = 99
OW2 = OW + C                 # + device Lam estimates = 115

_WIN = 2
_NW = T // _WIN                  # 256 DP windows

_prog_cache = {}
_targets_cache = {}

_BI = np.arange(128) // C              # lane -> local sample
_CI = np.arange(128) % C               # lane -> chunk

# static u8 blob columns (shipped once per targets, cached on device)
_U8_TRICS = 0        # [0:128]    strict block-upper-tri over lanes
_U8_IDENT = 128      # [128:256]  identity
_U8_ALLOW = 256      # [256:285]  allow2 per odd row
_U8_OHC = 285        # [285:301]  onehotC[p,k] = (c(p)==k)
_U8_MASK0 = 301      # [301:302]  mask_c0[p] = (c(p)==0)
_U8_OHBT = 302       # [302:430]  rows 0..7: onehotBT[b,p] = (b(p)==b)
U8W = 430


def _slot(l):
    return 0 if l % 2 == 0 else (l + 1) // 2


# --------------------------------------------------------------------------
# host-side prep
# --------------------------------------------------------------------------

def _prep_targets(targets):
    """Per-targets constants: fused gather index (lane layout) + the static
    part of the concatenated u8 blob for all 8 cores."""
    key = targets.tobytes()
    if key in _targets_cache:
        return _targets_cache[key]
    t2 = np.asarray(targets).reshape(B, S).astype(np.int64)
    ext = np.zeros((B, L), dtype=np.int64)
    ext[:, 1::2] = t2
    ext_m2 = np.zeros_like(ext)
    ext_m2[:, 2:] = ext[:, :-2]
    allow_odd = ((ext != BLANK) & (ext != ext_m2))[:, 3::2].astype(np.uint8)
    vrows = np.zeros((B, NS), np.int64)
    vrows[:, 1:] = t2                  # slot s>=1 -> label s-1; slot 0 = blank
    # flat-gather index producing the (b, chunk, slot, t') lane layout
    # directly:  idx[b,c,s,t'] = ((c*TC+t')*B + b)*V + vrows[b,s]
    tt = np.arange(T).reshape(C, TC)
    idx = ((tt[None, :, None, :] * B + np.arange(B)[:, None, None, None]) * V
           + vrows[:, None, :, None]).astype(np.int32)

    bi, ci = _BI, _CI
    same_b = bi[:, None] == bi[None, :]
    trics = (same_b & (ci[:, None] < ci[None, :])).astype(np.uint8)
    ident = np.eye(128, dtype=np.uint8)
    onehotC = (ci[:, None] == np.arange(C)[None, :]).astype(np.uint8)
    mask0 = (ci == 0).astype(np.uint8)[:, None]
    onehotBT = np.zeros((128, 128), np.uint8)
    onehotBT[:BLOC, :] = (np.arange(BLOC)[:, None] == bi[None, :])

    u8buf = np.zeros((NLANE, U8W), np.uint8)
    for k in range(NCORES):
        rs = slice(k * 128, (k + 1) * 128)
        u8buf[rs, _U8_TRICS:_U8_TRICS + 128] = trics
        u8buf[rs, _U8_IDENT:_U8_IDENT + 128] = ident
        u8buf[rs, _U8_ALLOW:_U8_ALLOW + 29] = \
            allow_odd[k * BLOC:(k + 1) * BLOC][_BI]
        u8buf[rs, _U8_OHC:_U8_OHC + C] = onehotC
        u8buf[rs, _U8_MASK0:_U8_MASK0 + 1] = mask0
        u8buf[rs, _U8_OHBT:_U8_OHBT + 128] = onehotBT
    out = {"idx": idx, "u8s": u8buf}
    _targets_cache.clear()
    _targets_cache[key] = out
    return out


def _host_prep_fn(lpf, idx):
    """Jitted XLA-CPU host prep: fused gather into lane layout,
    max-normalize, fp8 cast, windowed emission sums for the on-device Lam
    DP, and assembly of the transfer-ready concat arrays."""
    import jax.numpy as jnp

    A_lane = jnp.take(lpf, idx)                    # (B,C,NS,TC)
    m_lane = A_lane.max(axis=2)                    # (B,C,TC)
    Z_lane = A_lane - m_lane[:, :, None, :]
    z8 = Z_lane.astype(jnp.float8_e4m3)
    z8_cat = z8.reshape(NLANE, NS, TC)
    f32_cat = m_lane.reshape(NLANE, TC)
    return z8_cat, f32_cat


def _host_prep(log_probs, idx):
    if "fn" not in _prog_cache:
        _prog_cache["fn"] = jax.jit(_host_prep_fn)
    cpu = jax.devices("cpu")[0]
    with jax.default_device(cpu):
        z8_cat, f32_cat = _prog_cache["fn"](log_probs.reshape(-1), idx)
        return np.asarray(z8_cat), np.asarray(f32_cat)


# --------------------------------------------------------------------------
# device program (identical for all cores; per-core data differs)
# --------------------------------------------------------------------------

def _build_program():
    nc = bacc.Bacc(None)
    # consolidated inputs; u8s is static per targets and stays cached on
    # the device between calls (host passes the same jax.Array handle)
    d_z = nc.declare_dram_parameter("z", [128, NS, TC], F8, isOutput=False)
    d_f32 = nc.declare_dram_parameter("f32b", [128, TC], F32, isOutput=False)
    d_u8 = nc.declare_dram_parameter("u8s", [128, U8W], U8, isOutput=False)
    d_sel = nc.declare_dram_parameter("sel", [128, BLOC], U8, isOutput=False)
    out = nc.declare_dram_parameter("out", [BLOC, OW2], F32, isOutput=True)

    with tile.TileContext(nc) as tc:
        with (
            tc.tile_pool(name="consts", bufs=1) as consts,
            tc.tile_pool(name="rows", bufs=1) as rowsp,
            tc.tile_pool(name="work", bufs=3) as work,
            tc.tile_pool(name="gpool", bufs=3) as gpool,
            tc.tile_pool(name="gamp", bufs=2) as gamp,
            tc.tile_pool(name="ps", bufs=2, space="PSUM") as ps,
            tc.tile_pool(name="ps1", bufs=1, space="PSUM") as ps1,
        ):
            # ---- const loads (u8 -> f32 converts on the ACT engine) ----
            u8_all = consts.tile([128, U8W], U8)
            nc.sync.dma_start(out=u8_all[:], in_=d_u8[:])
            sb_trics = consts.tile([128, 128], F32)
            nc.scalar.copy(sb_trics[:], u8_all[:, 0:128])
            sb_ident = consts.tile([128, 128], F32)
            nc.scalar.copy(sb_ident[:], u8_all[:, 128:256])
            # tribias = (trics - 1) * 1e30  (0 where skip allowed, -1e30 else)
            sb_tribias = consts.tile([128, 128], F32)
            nc.vector.tensor_scalar(
                out=sb_tribias[:], in0=sb_trics[:], scalar1=1.0,
                scalar2=1e30,
                op0=mybir.AluOpType.subtract, op1=mybir.AluOpType.mult)
            sb_allow2 = consts.tile([128, 29], F32)
            nc.scalar.copy(sb_allow2[:], u8_all[:, _U8_ALLOW:_U8_ALLOW + 29])
            sel_u8 = consts.tile([128, BLOC], U8)
            nc.sync.dma_start(out=sel_u8[:], in_=d_sel[:])
            sb_sel = consts.tile([128, BLOC], F32)
            nc.scalar.copy(sb_sel[:], sel_u8[:])
            sb_ohc = consts.tile([128, C], F32)
            nc.scalar.copy(sb_ohc[:], u8_all[:, _U8_OHC:_U8_OHC + C])
            sb_mask0 = consts.tile([128, 1], F32)
            nc.scalar.copy(sb_mask0[:], u8_all[:, _U8_MASK0:_U8_MASK0 + 1])
            sb_ohbt = consts.tile([BLOC, 128], F32)
            nc.scalar.copy(sb_ohbt[:], u8_all[0:BLOC, _U8_OHBT:_U8_OHBT + 128])

            sb_f32 = consts.tile([128, TC], F32)
            nc.sync.dma_start(out=sb_f32[:], in_=d_f32[:])
            sb_ones = consts.tile([1, 128], F32)
            nc.vector.memset(sb_ones[:], 1.0)
            sb_zeros = consts.tile([128, TC], F32)
            nc.vector.memset(sb_zeros[:], 0.0)

            # ---- normalized emission lanes (host-gathered fp8) ----
            sb_z = consts.tile([128, NS, TC], F8)
            nc.sync.dma_start(out=sb_z[:], in_=d_z[:])

            # ---- lam-independent bulk first (short; keeps the vector/ACT
            # queues busy before the serial DP dominates them) ----
            sb_m = sb_f32[:, 0:TC]
            cumM = consts.tile([128, TC], F32)
            nc.vector.tensor_tensor_scan(
                out=cumM[:], data0=sb_m, data1=sb_zeros[:], initial=0.0,
                op0=mybir.AluOpType.add, op1=mybir.AluOpType.add)
            ps_baseM = ps1.tile([128, 1], F32, tag="bulk")
            nc.tensor.matmul(out=ps_baseM[:], lhsT=sb_trics[:],
                             rhs=cumM[:, TC - 1:TC], start=True, stop=True)
            sb_baseM = consts.tile([128, 1], F32)
            nc.scalar.copy(sb_baseM[:], ps_baseM[:])

            # per-slot chunk sums + exp(z), lam-independent
            sb_p = consts.tile([128, NS, TC], F32)
            sb_S = consts.tile([128, NS], F32)
            ps_lcs = ps1.tile([128, NS], F32, tag="lcs_all")
            GRP = 8
            for g0 in range(0, NS, GRP):
                g1 = min(g0 + GRP, NS)
                nc.vector.tensor_reduce(out=sb_S[:, g0:g1],
                                        in_=sb_z[:, g0:g1, :],
                                        axis=mybir.AxisListType.X,
                                        op=mybir.AluOpType.add)
                nc.scalar.activation(sb_p[:, g0:g1, :], sb_z[:, g0:g1, :],
                                     mybir.ActivationFunctionType.Exp)
                nc.tensor.matmul(out=ps_lcs[:, g0:g1], lhsT=sb_trics[:],
                                 rhs=sb_S[:, g0:g1], start=True, stop=True)

            # ---- on-device Lam DP (linear space, renormalized every
            # window; all steps on the DVE so the serial chain never pays
            # cross-engine semaphore hops) ----
            # window pair-products of exp(z): pp[lane, s, w'] =
            # p[., s, 2w'] * p[., s, 2w'+1]  (= exp of the window sum)
            WH = TC // _WIN
            pp = consts.tile([128, NS, WH], F32)
            nc.vector.tensor_tensor(out=pp[:], in0=sb_p[:, :, 0::2],
                                    in1=sb_p[:, :, 1::2],
                                    op=mybir.AluOpType.mult)
            # cross-partition reshape (b,c) lanes -> b rows (flat orders
            # of both access patterns coincide: (b, c, s, w'))
            zwB = consts.tile([BLOC, C, NS, WH], F32)
            nc.sync.dma_start(out=zwB[:], in_=pp[:])
            # expand slots -> lattice rows, clamp away exact zeros (an
            # all-zero window would make the renorm divide 0/0)
            PL = consts.tile([BLOC, C, WH, L], F32)
            for l in range(L):
                nc.scalar.copy(PL[:, :, :, l], zwB[:, :, _slot(l), :])
            nc.vector.tensor_scalar(out=PL[:], in0=PL[:], scalar1=1e-35,
                                    scalar2=None, op0=mybir.AluOpType.max)
            # state tiles: cols [0,1] stay 0 forever (shift-in boundary)
            stA = consts.tile([BLOC, 2 + L], F32)
            stB = consts.tile([BLOC, 2 + L], F32)
            nc.vector.memset(stA[:], 0.0)
            nc.vector.memset(stB[:], 0.0)
            nc.vector.memset(stA[:, 2:4], 1.0)
            dp_a = consts.tile([BLOC, L], F32)
            dp_b = consts.tile([BLOC, L], F32)
            rbuf = consts.tile([BLOC, _NW], F32)
            rinvb = consts.tile([BLOC, _NW], F32)
            zerw = consts.tile([BLOC, _NW], F32)
            nc.vector.memset(zerw[:], 0.0)
            cur, nxt = stA, stB
            for w in range(_NW):
                c_, wp = divmod(w, WH)
                nc.vector.tensor_tensor(out=dp_a[:], in0=cur[:, 2:2 + L],
                                        in1=cur[:, 1:1 + L],
                                        op=mybir.AluOpType.add)
                nc.vector.tensor_tensor(out=dp_a[:], in0=dp_a[:],
                                        in1=cur[:, 0:L],
                                        op=mybir.AluOpType.add)
                nc.vector.tensor_tensor(out=dp_b[:], in0=dp_a[:],
                                        in1=PL[:, c_, wp, :],
                                        op=mybir.AluOpType.mult)
                nc.vector.tensor_reduce(out=rbuf[:, w:w + 1], in_=dp_b[:],
                                        axis=mybir.AxisListType.X,
                                        op=mybir.AluOpType.max)
                nc.vector.reciprocal(rinvb[:, w:w + 1], rbuf[:, w:w + 1])
                nc.vector.tensor_scalar(out=nxt[:, 2:2 + L], in0=dp_b[:],
                                        scalar1=rinvb[:, w:w + 1],
                                        scalar2=None,
                                        op0=mybir.AluOpType.mult)
                cur, nxt = nxt, cur
            # levels: the scale actually applied is prod rinv_i, so
            # lam(w) = -sum_{i<=w} ln rinv_i, exactly; take chunk middles
            lnr = consts.tile([BLOC, _NW], F32)
            nc.scalar.activation(lnr[:], rinvb[:],
                                 mybir.ActivationFunctionType.Ln)
            cuml = consts.tile([BLOC, _NW], F32)
            nc.vector.tensor_tensor_scan(
                out=cuml[:], data0=lnr[:], data1=zerw[:], initial=0.0,
                op0=mybir.AluOpType.add, op1=mybir.AluOpType.add)
            lamB = consts.tile([BLOC, C], F32)
            nc.scalar.mul(lamB[:], cuml[:, WH // 2::WH], -1.0)

            # broadcast lamB (BLOC, C) to lane layout (128, 1):
            # lam[p] = lamB[b(p), c(p)] via one-hot matmul + masked reduce
            ps_lam = ps1.tile([128, C], F32, tag="ps_t")
            nc.tensor.matmul(out=ps_lam[:], lhsT=sb_ohbt[:], rhs=lamB[:],
                             start=True, stop=True)
            lam_sel = consts.tile([128, C], F32)
            nc.vector.tensor_tensor(out=lam_sel[:], in0=ps_lam[:],
                                    in1=sb_ohc[:], op=mybir.AluOpType.mult)
            sb_lam = consts.tile([128, 1], F32)
            nc.vector.tensor_reduce(out=sb_lam[:], in_=lam_sel[:],
                                    axis=mybir.AxisListType.X,
                                    op=mybir.AluOpType.add)
            # e0 seed column: exp(-lam) on chunk-0 lanes.  Mask lam BEFORE
            # the exp — exp(-lam) on far chunks overflows to inf and
            # inf*0 = NaN would poison the row scans.
            sb_e0 = consts.tile([128, TC], F32)
            nc.vector.memset(sb_e0[:], 0.0)
            e0t = consts.tile([128, 1], F32)
            nc.vector.tensor_tensor(out=e0t[:], in0=sb_lam[:],
                                    in1=sb_mask0[:], op=mybir.AluOpType.mult)
            nc.scalar.activation(e0t[:], e0t[:],
                                 mybir.ActivationFunctionType.Exp,
                                 scale=-1.0)
            nc.vector.tensor_tensor(out=sb_e0[:, 0:1], in0=e0t[:],
                                    in1=sb_mask0[:], op=mybir.AluOpType.mult)

            # ---- lam-dependent cumulants ----
            biasvec = consts.tile([128, NS], F32)
            msider = consts.tile([128, NS], F32)
            nc.vector.tensor_scalar(
                out=msider[:], in0=ps_lcs[:],
                scalar1=sb_lam[:], scalar2=None,
                op0=mybir.AluOpType.subtract)
            # tric = trics + I  =>  lam - tric@S = -(msider + S)
            nc.vector.tensor_tensor(
                out=biasvec[:], in0=msider[:],
                in1=sb_S[:], op=mybir.AluOpType.add)
            nc.vector.tensor_scalar(
                out=biasvec[:], in0=biasvec[:],
                scalar1=-1.0, scalar2=None, op0=mybir.AluOpType.mult)

            # ---- per-slot G transfer matrices ----
            def build_G(s, pool, tag):
                ps_t = ps1.tile([1, 128], F32, tag="ps_t")
                nc.tensor.transpose(out=ps_t[:], in_=msider[:, s:s + 1],
                                    identity=sb_ident[:])
                stg = work.tile([1, 128], F32, tag="stg")
                nc.scalar.copy(stg[:], ps_t[:])
                psG = ps.tile([128, 128], F32, tag="psG")
                nc.tensor.matmul(out=psG[:], lhsT=sb_ones[:],
                                 rhs=stg[:], start=True, stop=False)
                nc.tensor.matmul(out=psG[:], lhsT=sb_ident[:],
                                 rhs=sb_tribias[:], start=False, stop=True)
                Gt = pool.tile([128, 128], F32, tag=tag)
                nc.scalar.activation(Gt[:], psG[:],
                                     mybir.ActivationFunctionType.Exp,
                                     bias=biasvec[:, s:s + 1])
                return Gt

            G_blank = build_G(0, consts, "Gblank")

            # ---- lattice rows ----
            row_tiles = []
            gam_prev = {}
            for l in range(L):
                s = _slot(l)
                Gt = G_blank if s == 0 else build_G(s, gpool, "G")
                p_l = sb_p[:, s, :]
                if l == 0:
                    src_ap = sb_e0[:]
                elif l == 1:
                    srct = work.tile([128, TC], F32, tag="src")
                    nc.vector.tensor_add(out=srct[:],
                                         in0=row_tiles[0][:, 0:TC],
                                         in1=sb_e0[:])
                    src_ap = srct[:]
                elif l % 2 == 0:
                    src_ap = row_tiles[l - 1][:, 0:TC]
                else:
                    srct = work.tile([128, TC], F32, tag="src")
                    nc.vector.tensor_add(out=srct[:],
                                         in0=row_tiles[l - 1][:, 0:TC],
                                         in1=gam_prev[l - 2][:, 0:TC])
                    src_ap = srct[:]

                loc = work.tile([128, TC], F32, tag="loc")
                nc.vector.tensor_tensor_scan(
                    out=loc[:], data0=src_ap, data1=p_l, initial=0.0,
                    op0=mybir.AluOpType.add, op1=mybir.AluOpType.mult)
                xps = ps.tile([128, 1], F32, tag="xps")
                nc.tensor.matmul(out=xps[:], lhsT=Gt[:],
                                 rhs=loc[:, TC - 1:TC], start=True, stop=True)
                rowl = rowsp.tile([128, TC + 1], F32, tag=f"row{l}")
                nc.vector.tensor_tensor_scan(
                    out=rowl[:, 1:TC + 1], data0=src_ap, data1=p_l,
                    initial=xps[:, 0:1],
                    op0=mybir.AluOpType.add, op1=mybir.AluOpType.mult)
                nc.scalar.copy(rowl[:, 0:1], xps[:, 0:1])
                row_tiles.append(rowl)
                if l % 2 == 1 and l + 2 < L:
                    gaml = gamp.tile([128, TC + 1], F32, tag="gam")
                    nc.scalar.mul(gaml[:], rowl[:],
                                  sb_allow2[:, (l - 1) // 2:(l - 1) // 2 + 1])
                    gam_prev[l] = gaml

            # ---- outputs: one-hot matmul picks each sample's final lane
            # (exact: each PSUM sum has exactly one nonzero product) ----
            ps_out = ps1.tile([BLOC, OW], F32, tag="ps_out")
            nc.tensor.matmul(out=ps_out[:, 0:TC + 1], lhsT=sb_sel[:],
                             rhs=row_tiles[L - 2][:], start=True, stop=True)
            nc.tensor.matmul(out=ps_out[:, TC + 1:2 * TC + 2], lhsT=sb_sel[:],
                             rhs=row_tiles[L - 1][:], start=True, stop=True)
            nc.tensor.matmul(out=ps_out[:, 2 * TC + 2:2 * TC + 3],
                             lhsT=sb_sel[:], rhs=sb_baseM[:],
                             start=True, stop=True)
            nc.tensor.matmul(out=ps_out[:, 2 * TC + 3:OW], lhsT=sb_sel[:],
                             rhs=cumM[:], start=True, stop=True)
            sb_out = consts.tile([BLOC, OW2], F32)
            nc.scalar.copy(sb_out[:, 0:OW], ps_out[:])
            nc.scalar.copy(sb_out[:, OW:OW2], lamB[:])
            nc.sync.dma_start(out=out[:], in_=sb_out[:])
    nc.finalize()
    return nc


# --------------------------------------------------------------------------
# once-compiled PJRT executable (the per-call path is just: feed arrays)
# --------------------------------------------------------------------------

def _build_compiled():
    """Lower + compile the 8-core shard_map'd bass_exec once; returns
    (compiled, in_names, out_shapes)."""
    from jax.sharding import Mesh, PartitionSpec
    from jax.experimental.shard_map import shard_map
    import ml_dtypes

    nc = _build_program()
    bass2jax.install_neuronx_cc_hook()
    assert nc.dbg_addr is None
    partition_name = (nc.partition_id_tensor.name
                      if nc.partition_id_tensor else None)

    in_names, out_names, out_avals = [], [], []
    in_shapes = {}
    for alloc in nc.m.functions[0].allocations:
        if not isinstance(alloc, mybir.MemoryLocationSet):
            continue
        name = alloc.memorylocations[0].name
        if alloc.kind == "ExternalInput":
            if name != partition_name:
                in_names.append(name)
                in_shapes[name] = (tuple(alloc.tensor_shape),
                                   mybir.dt.np(alloc.dtype))
        elif alloc.kind == "ExternalOutput":
            out_names.append(name)
            out_avals.append(jax.core.ShapedArray(
                tuple(alloc.tensor_shape), mybir.dt.np(alloc.dtype)))
    n_params = len(in_names)
    n_outs = len(out_avals)
    all_names = in_names + out_names
    if partition_name is not None:
        all_names = all_names + [partition_name]
    donate = tuple(range(n_params, n_params + n_outs))

    def _body(*args):
        operands = list(args)
        if partition_name is not None:
            operands.append(bass2jax.partition_id_tensor())
        outs = bass2jax._bass_exec_p.bind(
            *operands, out_avals=tuple(out_avals), in_names=tuple(all_names),
            out_names=tuple(out_names), lowering_input_output_aliases=(),
            sim_require_finite=True, sim_require_nnan=True, nc=nc)
        return tuple(outs)

    devices = jax.devices()[:NCORES]
    mesh = Mesh(np.asarray(devices), ("core",))
    in_specs = (PartitionSpec("core"),) * (n_params + n_outs)
    out_specs = (PartitionSpec("core"),) * n_outs
    sharded = jax.jit(
        shard_map(_body, mesh=mesh, in_specs=in_specs, out_specs=out_specs,
                  check_rep=False),
        donate_argnums=donate, keep_unused=True)

    def _np_dtype(dt):
        return (ml_dtypes.float8_e4m3 if dt == np.dtype(ml_dtypes.float8_e4m3fn)
                or "float8" in str(dt) else dt)

    dummy_in = [np.zeros((NCORES * in_shapes[nm][0][0],
                          *in_shapes[nm][0][1:]),
                         _np_dtype(in_shapes[nm][1])) for nm in in_names]
    out_shapes = [((NCORES * a.shape[0], *a.shape[1:]), a.dtype)
                  for a in out_avals]
    dummy_zero = [np.zeros(s, dt) for s, dt in out_shapes]
    compiled = sharded.lower(*dummy_in, *dummy_zero).compile()
    return compiled, in_names, out_shapes


def _get_compiled():
    if "compiled" not in _prog_cache:
        _prog_cache["compiled"] = _build_compiled()
    return _prog_cache["compiled"]


# --------------------------------------------------------------------------
# entry point
# --------------------------------------------------------------------------

def kernel(log_probs, targets, input_lengths, target_lengths):
    log_probs = np.asarray(log_probs, dtype=np.float32)
    targets = np.asarray(targets)
    input_lengths = np.asarray(input_lengths).astype(np.int64)
    target_lengths = np.asarray(target_lengths)

    tc_entry = _prep_targets(targets)
    idx = tc_entry["idx"]
    compiled, in_names, out_shapes = _get_compiled()

    # targets-static u8 blob: push to the devices once and reuse the
    # resident jax.Array on every later call (skips ~0.45MB/call of
    # tunnel transfer)
    if "u8s_dev" not in tc_entry:
        try:
            sh = compiled.input_shardings[0][in_names.index("u8s")]
            tc_entry["u8s_dev"] = jax.device_put(tc_entry["u8s"], sh)
        except Exception:
            tc_entry["u8s_dev"] = tc_entry["u8s"]
    u8s = tc_entry["u8s_dev"]

    # fused gather straight into the (b, chunk, slot, t') lane layout,
    # plus normalization and fp8 cast — one jitted XLA-CPU call emitting
    # the transfer-ready concat arrays
    z8_cat, f32_cat = _host_prep(log_probs, idx)

    # final-frame lane selection per sample (host knows input_lengths)
    tE = input_lengths - 1
    cb, tb = tE // TC, tE % TC
    selbuf = np.zeros((NLANE, BLOC), np.uint8)
    selbuf[np.arange(B) * C + cb, np.arange(B) % BLOC] = 1

    feed = {"z": z8_cat, "f32b": f32_cat, "u8s": u8s, "sel": selbuf}
    args = [feed[nm] for nm in in_names]
    zz = [np.zeros(s, dt) for s, dt in out_shapes]
    (out_arr,) = compiled(*args, *zz)

    # host-side: per-sample loss extraction + mean (the "all-reduce")
    o = np.asarray(out_arr).astype(np.float64)            # (B, OW2)
    bb = np.arange(B)
    j = 1 + tb
    A2 = o[bb, j] + o[bb, TC + 1 + j]
    lnorm = o[:, 2 * TC + 2] + o[bb, 2 * TC + 3 + tb] + o[bb, OW + cb]
    with np.errstate(divide="ignore", invalid="ignore"):
        losses = -(np.log(A2) + lnorm)
    bad = (A2 <= 0) | ~np.isfinite(losses) | (losses >= 1e29)
    losses[bad] = 0.0
    result = np.float32(np.mean((losses / target_lengths.astype(np.float64))
                                .astype(np.float32)))
    return np.asarray(result, dtype=np.float32)
